# revision 24
# baseline (speedup 1.0000x reference)
"""BestBuddyLoss Trainium2 kernel v2 (8-core data parallel).

Per image: q = [unfold(gt) | unfold(down2(gt)) | unfold(down4(gt))] padded to
3072 cols; argmin_j score(i,j) == argmax_j <p1_i+p2_i, q_j> - |q_j|^2, done as
K=33 f32r matmuls (27 data rows + 5 zero + bias row 32).  Logical column
j = 2k + b: even columns (b=0) stream through stride-2 matmuls into PSUM
(3 banks flat, [128,1536]); odd columns land in transit PSUM banks and are
copied to SBUF by ACT.  One 2-stream custom DVE op per i-tile scans
(A_k, B_k) pairs and emits argmax j = 2*Idx + (A<B) via running-max records
(record j's are monotone, so accum max = last record = argmax).
Tail: PMT permute -> wrapped i16 idx -> ap_gather q[j*] -> L1 reduce.

Engine schedule: image0 prep on DVE/ACT (head-critical), image1 x-path and
lhs build on Pool overlapping main(0); bias(1) slots between the mains.
"""

import sys

sys.path.insert(0, "/opt/trn_rl_repo")

import numpy as np

import concourse.bacc as bacc
import concourse.mybir as mybir
import concourse.tile as tile
from concourse.bass_utils import run_bass_kernel_spmd

# ---------------- problem constants (hardcoded) ----------------
B_FULL = 16
NCORES = 8
B_LOC = B_FULL // NCORES       # images per core
C, H, W = 3, 144, 144
G = 48                         # patch grid (144/3)
NI = G * G                     # 2304 query patches
D = 27                         # C*3*3
NQ = NI + (G // 2) ** 2 + (G // 4) ** 2  # 3024
NQP = 3072                     # padded j space (6*512)
KD = 33                        # contraction: 27 data + 5 pad + bias row 32
KZ = 32                        # bias row partition (32-aligned)
IT = 128
NIT = NI // IT                 # 18
HK = NQP // 2                  # 1536 pair count per tile
CH = 512
NCH = HK // CH                 # 3 chunks per half
PADBIAS = -1.0e30
CUBIC_W = np.array([-0.09375, 0.59375, 0.59375, -0.09375], dtype=np.float32)

F32 = mybir.dt.float32
F32R = mybir.dt.float32r
I16 = mybir.dt.int16
ADD = mybir.AluOpType.add
SUB = mybir.AluOpType.subtract
MUL = mybir.AluOpType.mult
ABS = mybir.ActivationFunctionType.Abs
SQ = mybir.ActivationFunctionType.Square

# ---------------- custom DVE op: 2-stream pair argmax ------------------
from concourse.dve_spec import Spec, Src0, Src1, C2, Zero, scan, AluOp, maxx, lower
from concourse.dve_uop import DveOpSpec
import concourse.dve_ops as dve_ops
from concourse.dve_ops import DveOp


def _pair_argmax_ref(in0, in1, c0, c1, c2):
    m = np.maximum(in0, in1)
    run = np.maximum.accumulate(m, axis=-1)
    n = in0.shape[-1]
    j = np.float32(c2) * np.arange(n, dtype=np.float32)[None, :] + (in0 < in1)
    out = (m >= run) * j
    acc = out.reshape(out.shape[0], -1).max(axis=-1, keepdims=True)
    return out.astype(np.float32), acc.astype(np.float32)


def _register_pair_op():
    name = "ANT_PAIR_ARGMAX"
    if name in dve_ops._SUB_OPCODE_FOR_NAME:
        return next(op for op in dve_ops.OPS if op.name == name)
    m = maxx(Src0, Src1)
    two_idx = scan(AluOp.ADD, C2, init=Zero - C2)   # 0, 2, 4, ... (C2=2.0)
    j = two_idx + (Src0 < Src1)
    body = (m >= scan(AluOp.MAX, m)) * j
    spec = Spec(body=body, accum=maxx, reference=_pair_argmax_ref)
    opcode = dve_ops._CUSTOM_DVE_ROW_BASE + len(dve_ops.OPS)
    shas = {v: DveOpSpec(name=name, opcode=opcode, uops=lower(spec, ver=v),
                         rd1_en=True).sha(v) for v in ("v3", "v4")}
    op = DveOp(name, spec, subdim=False, uops_sha=shas)
    dve_ops.OPS.append(op)
    dve_ops._SUB_OPCODE_FOR_NAME[name] = opcode
    dve_ops.CUSTOM_DVE_SPECS[name] = spec
    return op


PAIR_OP = _register_pair_op()


def _absdiff_acc_ref(in0, in1, c0, c1, c2):
    out = np.abs(in0 - in1).astype(np.float32)
    acc = out.reshape(out.shape[0], -1).sum(axis=-1, keepdims=True)
    return out, acc.astype(np.float32)


def _register_abs_op():
    name = "ANT_ABSDIFF_ACC"
    if name in dve_ops._SUB_OPCODE_FOR_NAME:
        return next(op for op in dve_ops.OPS if op.name == name)
    body = maxx(Src0 - Src1, Src1 - Src0)
    spec = Spec(body=body, accum=AluOp.ADD, reference=_absdiff_acc_ref)
    opcode = dve_ops._CUSTOM_DVE_ROW_BASE + len(dve_ops.OPS)
    shas = {v: DveOpSpec(name=name, opcode=opcode, uops=lower(spec, ver=v),
                         rd1_en=True).sha(v) for v in ("v3", "v4")}
    op = DveOp(name, spec, subdim=False, uops_sha=shas)
    dve_ops.OPS.append(op)
    dve_ops._SUB_OPCODE_FOR_NAME[name] = opcode
    dve_ops.CUSTOM_DVE_SPECS[name] = spec
    return op


ABS_OP = _register_abs_op()

# ---------------- host-side constants ---------------------------------


def _down_matrix(n, f):
    """M[h, i]: out[i] = sum_h M[h, i] * in[h]  (torch bicubic, offset t=.5)."""
    out_n = n // f
    M = np.zeros((n, out_n), dtype=np.float32)
    for i in range(out_n):
        base = f * i + (f // 2 - 1)
        for a in range(4):
            h = min(max(base + a - 1, 0), n - 1)
            M[h, i] += CUBIC_W[a]
    return M


def _perm_matrices():
    """PMT[:, m*128 + r]: one-hot at row (m*16 + r%16) -> out_m = Pm @ v."""
    P = np.zeros((128, 8 * 128), dtype=np.float32)
    for m in range(8):
        for r in range(128):
            P[m * 16 + r % 16, m * 128 + r] = 1.0
    return P


def make_consts():
    padq = np.zeros((KD - D, NQP), dtype=np.float32)
    padq[KZ - D, NQ:NQP] = PADBIAS
    padl = np.zeros((KD - D, NI), dtype=np.float32)
    padl[KZ - D, :] = 1.0
    return {
        "cd2": np.ascontiguousarray(_down_matrix(H, 2)),  # [144, 72]
        "cd4": np.ascontiguousarray(_down_matrix(H, 4)),  # [144, 36]
        "idn": np.eye(128, dtype=np.float32),
        "pmt": _perm_matrices(),
        "neg1": np.full((D, 1), -1.0, dtype=np.float32),
        "ones128": np.ones((128, 1), dtype=np.float32),
        "padq": padq,
        "padl": padl,
    }


# ---------------- kernel construction ---------------------------------


def build_nc(debug=False):
    nc = bacc.Bacc("TRN2", target_bir_lowering=False)

    x_d = nc.dram_tensor("x", [B_LOC, C, H, W], F32, kind="ExternalInput")
    gt_d = nc.dram_tensor("gt", [B_LOC, C, H, W], F32, kind="ExternalInput")
    cd2_d = nc.dram_tensor("cd2", [H, 72], F32, kind="ExternalInput")
    cd4_d = nc.dram_tensor("cd4", [H, 36], F32, kind="ExternalInput")
    idn_d = nc.dram_tensor("idn", [128, 128], F32, kind="ExternalInput")
    pmt_d = nc.dram_tensor("pmt", [128, 8 * 128], F32, kind="ExternalInput")
    neg1_d = nc.dram_tensor("neg1", [D, 1], F32, kind="ExternalInput")
    ones_d = nc.dram_tensor("ones128", [128, 1], F32, kind="ExternalInput")
    padq_d = nc.dram_tensor("padq", [KD - D, NQP], F32R, kind="ExternalInput")
    padl_d = nc.dram_tensor("padl", [KD - D, NI], F32R, kind="ExternalInput")
    d2_d = nc.dram_tensor("scr_d2", [B_LOC, C, 72, 72], F32, kind="Internal")
    d4_d = nc.dram_tensor("scr_d4", [B_LOC, C, 36, 36], F32, kind="Internal")
    loss_d = nc.dram_tensor("loss", [D, 2 * B_LOC], F32,
                            kind="ExternalOutput")
    dbg = {}
    if debug:
        dbg["rr"] = nc.dram_tensor("dbg_rr", [B_LOC, KD, NQP], F32, kind="ExternalOutput")
        dbg["lr"] = nc.dram_tensor("dbg_lr", [B_LOC, KD, NI], F32, kind="ExternalOutput")
        dbg["p1t"] = nc.dram_tensor("dbg_p1t", [B_LOC, D, NI], F32, kind="ExternalOutput")
        dbg["idxf"] = nc.dram_tensor("dbg_idxf", [B_LOC, 128, NIT], F32, kind="ExternalOutput")
        dbg["sel"] = nc.dram_tensor("dbg_sel", [B_LOC, 32, NI], F32, kind="ExternalOutput")

    with tile.TileContext(nc) as tc:
        with (
            tc.tile_pool(name="consts", bufs=1) as cpool,
            tc.tile_pool(name="stageA", bufs=2) as apool,
            tc.tile_pool(name="stageB", bufs=3) as bpool,
            tc.tile_pool(name="stageD", bufs=1) as dapool,
            tc.tile_pool(name="prep", bufs=2) as ppool,
            tc.tile_pool(name="persist", bufs=2) as spool,
            tc.tile_pool(name="scoreB", bufs=3) as scpool,
            tc.tile_pool(name="small", bufs=2) as smpool,
            tc.tile_pool(name="psA", bufs=2, space="PSUM") as psA,
            tc.tile_pool(name="psB", bufs=2, space="PSUM") as psB,
        ):
            cd2a_f = cpool.tile([128, 72], F32, tag="cd2af")
            cd2b_f = cpool.tile([16, 72], F32, tag="cd2bf")
            cd4a_f = cpool.tile([128, 36], F32, tag="cd4af")
            cd4b_f = cpool.tile([16, 36], F32, tag="cd4bf")
            cd2a = cpool.tile([128, 72], F32R, tag="cd2a")
            cd2b = cpool.tile([16, 72], F32R, tag="cd2b")
            cd4a = cpool.tile([128, 36], F32R, tag="cd4a")
            cd4b = cpool.tile([16, 36], F32R, tag="cd4b")
            idn_t = cpool.tile([128, 128], F32, tag="idn")
            pmt_t = cpool.tile([128, 8 * 128], F32, tag="pmt")
            neg1_t = cpool.tile([D, 1], F32, tag="neg1")
            neg1r = cpool.tile([D, 1], F32R, tag="neg1r")
            ones_t = cpool.tile([128, 1], F32, tag="ones")

            junk = cpool.tile([128, HK], F32, tag="junk")
            part = cpool.tile([128, 2 * B_LOC], F32, tag="part")

            def shuffle(eng, dst_v, src_v):
                if eng is nc.scalar:
                    eng.copy(dst_v, src_v)
                else:
                    eng.tensor_copy(dst_v, src_v)

            def unfold_big(dram_plane_ap, dst_ap, eng, round_f32r, nm):
                """dram [C, 144, 144] -> dst [27, 2304]; at loads on the SP
                queue (ready at t=0), shuffle + out-DMA on the shuffling
                engine's queue (no SP head-of-line blocking)."""
                g, gh = G, G // 2
                szh = 3 * gh * g
                for hf in range(2):
                    at = apool.tile([9, szh], F32, tag="A", name=f"at_{nm}{hf}")
                    for c in range(C):
                        src = dram_plane_ap[c].rearrange(
                            "(gi r) w -> r gi w", r=3
                        )[:, hf * gh:(hf + 1) * gh, :]
                        nc.sync.dma_start(at[3 * c:3 * c + 3, :], src)
                    bt = bpool.tile([9, szh], F32R if round_f32r else F32,
                                    tag="B", name=f"bt_{nm}{hf}")
                    rearr = at[:, :].rearrange(
                        "p (gi gj s) -> p s gi gj", gi=gh, gj=g, s=3)
                    bt_v = bt[:, :].rearrange(
                        "p (s gi gj) -> p s gi gj", s=3, gi=gh)
                    shuffle(eng, bt_v, rearr)
                    dma_eng = nc.gpsimd if eng is nc.gpsimd else nc.scalar
                    fh = gh * g
                    dst_half = dst_ap[:, hf * fh:(hf + 1) * fh]
                    if round_f32r:
                        dma_eng.dma_start(dst_half.bitcast(F32),
                                          bt[:, :].bitcast(F32))
                    else:
                        dma_eng.dma_start(dst_half, bt[:, :])

            def unfold_small(dsc_d, f, dst_ap, eng, nm):
                """downsampled DRAM scratch [C, n, n] ->
                dst [27, gsub*gsub]."""
                n = H // f
                gsub = n // 3
                sz = 3 * gsub * gsub
                dat = dapool.tile([9, 3 * 24 * 24], F32, tag="DA",
                                  name=f"dat_{nm}")
                dma_eng = nc.gpsimd if eng is nc.gpsimd else nc.scalar
                for c in range(C):
                    src = dsc_d[c].rearrange("(gi r) w -> r gi w", r=3)
                    dma_eng.dma_start(
                        dat[3 * c:3 * c + 3, 0:gsub * n].rearrange(
                            "p (gi w) -> p gi w", gi=gsub), src)
                bt = bpool.tile([9, 3 * (G // 2) * G], F32R, tag="B",
                                name=f"bt_{nm}")
                rearr = dat[:, 0:sz].rearrange(
                    "p (gi gj s) -> p s gi gj", gi=gsub, gj=gsub, s=3)
                bt_v = bt[:, 0:sz].rearrange(
                    "p (s gi gj) -> p s gi gj", s=3, gi=gsub)
                shuffle(eng, bt_v, rearr)
                dma_eng.dma_start(dst_ap.bitcast(F32),
                                  bt[:, 0:sz].bitcast(F32))

            def downsample(b, f, ga, gb, out_dram, psp=None, pstag="psB",
                           ceng=None):
                """gt[b] --bicubic/f--> out_dram [C, n, n]."""
                psp = psp or psB
                def ptile():
                    t = psp.tile([128, NCH, CH] if psp is psA else [128, CH],
                                 F32, tag=pstag)
                    return t[:, 0, :] if psp is psA else t
                n = H // f
                cda = cd2a if f == 2 else cd4a
                cdb = cd2b if f == 2 else cd4b
                cdaf = cd2a_f if f == 2 else cd4a_f
                cdbf = cd2b_f if f == 2 else cd4b_f
                ghp = ptile()
                nc.tensor.matmul(ghp[0:n, 0:C * H], cda[:, 0:n], ga[:],
                                 start=True, stop=False)
                nc.tensor.matmul(ghp[0:n, 0:C * H], cdb[:, 0:n], gb[:],
                                 start=False, stop=True)
                ceng = ceng or nc.scalar
                def pcopy(dst, srcp):
                    if ceng is nc.scalar:
                        ceng.copy(dst, srcp)
                    else:
                        ceng.tensor_copy(dst, srcp)
                gh = ppool.tile([72, C * H], F32, tag="gh", bufs=1)
                pcopy(gh[0:n, :], ghp[0:n, 0:C * H])
                gh3 = gh[:].rearrange("i (c w) -> i c w", c=C)
                ghta = ppool.tile([128, C * 72], F32R, tag="ghta")
                ghtb = ppool.tile([16, C * 72], F32R, tag="ghtb")
                ghta3 = ghta[:].rearrange("w (c i) -> w c i", c=C)
                ghtb3 = ghtb[:].rearrange("w (c i) -> w c i", c=C)
                for c in range(C):
                    tp = ptile()
                    nc.tensor.transpose(tp[0:128, 0:n], gh3[0:n, c, 0:128],
                                        idn_t[0:n, 0:n])
                    pcopy(ghta3[:, c, 0:n], tp[0:128, 0:n])
                    tp2 = ptile()
                    nc.tensor.transpose(tp2[0:16, 0:n],
                                        gh3[0:n, c, 128:144], idn_t[0:n, 0:n])
                    pcopy(ghtb3[:, c, 0:n], tp2[0:16, 0:n])
                g2 = ppool.tile([72, C * 72], F32, tag=f"g2_{f}")
                g23 = g2[:].rearrange("i (c j) -> i c j", c=C)
                for c in range(C):
                    op = ptile()
                    nc.tensor.matmul(op[0:n, 0:n], ghta3[:, c, 0:n],
                                     cda[:, 0:n], start=True, stop=False)
                    nc.tensor.matmul(op[0:n, 0:n], ghtb3[:, c, 0:n],
                                     cdb[:, 0:n], start=False, stop=True)
                    pcopy(g23[0:n, c, 0:n], op[0:n, 0:n])
                out_ap = out_dram.rearrange("c h w -> h c w")
                nc.scalar.dma_start(out_ap, g23[0:n, :, 0:n])

            rrs = [None] * B_LOC
            lrs = [None] * B_LOC
            p1ts = [None] * B_LOC
            idxf = [None] * B_LOC
            widxs = [None] * B_LOC
            sels = [None] * B_LOC

            def prep_head(b, eng):
                """pads, downsamples, gt/d2/d4 unfolds -> rr q rows."""
                rr = spool.tile([KD, NQP], F32R, tag="rr")
                lr = spool.tile([KD, NI], F32R, tag="lr")
                p1t = spool.tile([D, NI], F32, tag="p1t")
                rrs[b], lrs[b], p1ts[b] = rr, lr, p1t
                idxf[b] = smpool.tile([128, NIT], F32, tag="idxf",
                                      name=f"idxf{b}")
                widxs[b] = smpool.tile([128, 8 * NIT], I16, tag="widx",
                                       name=f"widx{b}")
                sels[b] = smpool.tile([32, NI], F32, tag="sel", bufs=1,
                                      name=f"sel{b}")
                # pad rows 27:33 (zeros + bias/ones rows) via one DMA each
                nc.gpsimd.dma_start(rr[D:KD, :], padq_d[:])
                nc.gpsimd.dma_start(lr[D:KD, :], padl_d[:])
                nc.gpsimd.memset(rr[0:KZ, NQ:NQP].bitcast(F32), 0.0)

                ga = ppool.tile([128, C * H], F32, tag="gplane_a", bufs=1)
                gb = ppool.tile([16, C * H], F32, tag="gplane_b", bufs=1)
                gar = ppool.tile([128, C * H], F32R, tag="gplane_ar", bufs=1)
                gbr = ppool.tile([16, C * H], F32R, tag="gplane_br", bufs=1)
                gsrc = gt_d[b].rearrange("c h w -> h c w")
                nc.sync.dma_start(ga[:], gsrc[0:128])
                nc.sync.dma_start(gb[:], gsrc[128:144])
                ceng = nc.vector if b == 0 else nc.gpsimd
                ceng.tensor_copy(gar[:], ga[:])
                ceng.tensor_copy(gbr[:], gb[:])
                psp = psA if b == 0 else psB
                downsample(b, 4, gar, gbr, d4_d[b], psp=psp,
                           pstag="psA" if b == 0 else "psB",
                           ceng=None if b == 0 else nc.vector)
                downsample(b, 2, gar, gbr, d2_d[b], psp=psp,
                           pstag="psA" if b == 0 else "psB",
                           ceng=None if b == 0 else nc.vector)
                unfold_big(gt_d[b], rr[0:D, 0:NI], eng, True, f"gt{b}")

            def prep_dsmall(b, eng):
                rr = rrs[b]
                unfold_small(d4_d[b], 4, rr[0:D, NI + 576:NQ], eng, f"d4{b}")
                unfold_small(d2_d[b], 2, rr[0:D, NI:NI + 576], eng, f"d2{b}")

            qsqs = [None] * B_LOC

            def get_qsq(b):
                if qsqs[b] is None:
                    qsqs[b] = ppool.tile([D, NQ], F32R, tag="qsq", bufs=1,
                                         name=f"qsq{b}")
                return qsqs[b]

            def qsq_sq(b, part_, use_pool):
                rr = rrs[b]
                qsq = get_qsq(b)
                lo, hi = (0, 2016) if part_ == 0 else (2016, NQ)
                if use_pool == "pool":
                    nc.gpsimd.tensor_tensor(qsq[:, lo:hi],
                                            rr[0:D, lo:hi].bitcast(F32),
                                            rr[0:D, lo:hi].bitcast(F32),
                                            op=MUL)
                elif use_pool == "dve":
                    nc.vector.tensor_tensor(qsq[:, lo:hi],
                                            rr[0:D, lo:hi].bitcast(F32),
                                            rr[0:D, lo:hi].bitcast(F32),
                                            op=MUL)
                else:
                    nc.scalar.activation(qsq[:, lo:hi],
                                         rr[0:D, lo:hi].bitcast(F32), SQ)

            def qsq_bias_row(b, part_, psp=None, pstag="psB", split=False):
                psp = psp or psB
                rr = rrs[b]
                qsq = get_qsq(b)
                lo, hi = (0, 2016) if part_ == 0 else (2016, NQ)
                for jt in range(lo // 504, hi // 504):
                    if psp is psA:
                        pt = psp.tile([128, NCH, CH], F32, tag=pstag)
                        bnp = pt[:, 0, :]
                    else:
                        bnp = psp.tile([128, CH], F32, tag=pstag)
                    nc.tensor.matmul(bnp[0:1, 0:504], neg1r[:],
                                     qsq[:, jt * 504:(jt + 1) * 504])
                    sl = rr[KZ:KZ + 1, jt * 504:(jt + 1) * 504]
                    if split and jt % 2 == 1:
                        nc.vector.tensor_copy(sl, bnp[0:1, 0:504])
                    else:
                        nc.scalar.copy(sl, bnp[0:1, 0:504])
                if debug and part_ == 1:
                    nc.sync.dma_start(dbg["rr"][b], rr[:, :].bitcast(F32))

            def qsq_bias(b, use_pool, part_, psp=None, pstag="psB"):
                qsq_sq(b, part_, use_pool)
                qsq_bias_row(b, part_, psp=psp, pstag=pstag)

            def prep_x(b, eng_shuf, eng_add):
                """p1t = unfold(x); lr rows 0:27 = p1 + p2."""
                unfold_big(x_d[b], p1ts[b][:, :], eng_shuf, False, f"x{b}")
                eng_add.tensor_tensor(lrs[b][0:D, :], p1ts[b][:, :],
                                      rrs[b][0:D, 0:NI].bitcast(F32), op=ADD)
                if debug:
                    nc.sync.dma_start(dbg["lr"][b], lrs[b][:, :].bitcast(F32))
                    nc.sync.dma_start(dbg["p1t"][b], p1ts[b][:, :])

            def main(b, hook=None):
                lr, rr = lrs[b], rrs[b]
                rv = rr[:, :].rearrange("p (k two) -> p k two", two=2)
                for t in range(NIT):
                    if hook is not None:
                        hook(t)
                    lrt = lr[:, t * IT:(t + 1) * IT]
                    scB = scpool.tile([128, HK], F32, tag="scB")
                    for c in range(NCH):
                        pb = psB.tile([128, CH], F32, tag="psB")
                        nc.tensor.matmul(pb[:, :], lrt,
                                         rv[:, CH * c:CH * (c + 1), 1])
                        nc.scalar.copy(scB[:, CH * c:CH * (c + 1)],
                                       pb[:, :])
                    pa = psA.tile([128, NCH, CH], F32, tag="psA")
                    for c in range(NCH):
                        nc.tensor.matmul(pa[:, c, :], lrt,
                                         rv[:, CH * c:CH * (c + 1), 0])
                    flatA = pa[:, :, :].rearrange("p a b -> p (a b)")
                    nc.vector._custom_dve(
                        PAIR_OP, out=junk[:], in0=flatA[:, :], in1=scB[:, :],
                        accum_out=idxf[b][:, t:t + 1], imm2=2.0,
                    )
                if debug:
                    nc.sync.dma_start(dbg["idxf"][b], idxf[b][:, :])

            HT = NIT // 2
            HNI = NI // 2

            def tail_idx(b, h):
                """PMT permute + wrapped i16 idx for tiles [h*9,(h+1)*9)."""
                wp = psB.tile([128, CH], F32, tag="psB",
                              name=f"pswp{b}_{h}")
                wp3 = wp[0:128, 0:8 * HT].rearrange("p (m t) -> p m t", m=8)
                for m in range(8):
                    nc.tensor.matmul(
                        wp3[:, m, :], pmt_t[:, m * 128:(m + 1) * 128],
                        idxf[b][:, h * HT:(h + 1) * HT],
                    )
                w3 = widxs[b][:].rearrange("p (t m) -> p t m", t=NIT)
                nc.vector.tensor_copy(
                    w3[:, h * HT:(h + 1) * HT, :],
                    wp3[:, :, :].rearrange("p m t -> p t m"))

            def tail_gather(b):
                sel = sels[b]
                nc.gpsimd.ap_gather(
                    sel[:, :].rearrange("p (n d) -> p n d", d=1),
                    rrs[b][0:32, :].bitcast(F32).rearrange(
                        "p (n d) -> p n d", d=1),
                    widxs[b][0:32, :],
                    channels=32, num_elems=NQP, d=1, num_idxs=NI,
                )
                if debug:
                    nc.sync.dma_start(dbg["sel"][b], sel[:, :])
                    nc.sync.dma_start(dbg["idxf"][b], idxf[b][:, :])

            def tail_abs(b, h):
                nc.vector._custom_dve(
                    ABS_OP, out=junk[0:D, 0:HNI],
                    in0=sels[b][0:D, h * HNI:(h + 1) * HNI],
                    in1=p1ts[b][:, h * HNI:(h + 1) * HNI],
                    accum_out=part[0:D, 2 * b + h:2 * b + h + 1],
                    imm2=0.0,
                )

            # ---------------- orchestration ----------------
            # const loads on the ACT hwdge queue so image-0's at-loads own
            # the first SP-queue HWDGE slots
            nc.scalar.dma_start(cd4a_f[:], cd4_d[0:128, :])
            nc.scalar.dma_start(cd4b_f[:], cd4_d[128:144, :])
            nc.scalar.dma_start(cd2a_f[:], cd2_d[0:128, :])
            nc.scalar.dma_start(cd2b_f[:], cd2_d[128:144, :])
            nc.scalar.dma_start(idn_t[:], idn_d[:])
            nc.scalar.dma_start(neg1_t[:], neg1_d[:])
            nc.scalar.dma_start(ones_t[:], ones_d[:])
            nc.vector.tensor_copy(cd4a[:], cd4a_f[:])
            nc.vector.tensor_copy(cd4b[:], cd4b_f[:])
            nc.vector.tensor_copy(cd2a[:], cd2a_f[:])
            nc.vector.tensor_copy(cd2b[:], cd2b_f[:])
            nc.vector.tensor_copy(neg1r[:], neg1_t[:])
            prep_head(0, nc.vector)    # image0 on DVE (head-critical)
            nc.scalar.dma_start(pmt_t[:], pmt_d[:])
            prep_x(0, nc.vector, nc.vector)
            prep_dsmall(0, nc.vector)
            qsq_bias(0, use_pool="act", part_=0, psp=psA, pstag="psA")
            qsq_bias(0, use_pool="act", part_=1, psp=psA, pstag="psA")
            prep_head(1, nc.gpsimd)    # image1 on Pool; ds copies on DVE
            prep_x(1, nc.gpsimd, nc.gpsimd)
            prep_dsmall(1, nc.gpsimd)

            qsq_sq(1, 0, use_pool="pool")
            qsq_sq(1, 1, use_pool="pool")
            main(0)
            qsq_bias_row(1, 0, split=True)
            qsq_bias_row(1, 1, split=True)
            # image-0 tail (gather overlaps main(1) on Pool)
            tail_idx(0, 0)
            tail_idx(0, 1)
            tail_gather(0)
            # image-1 idx/gather chain emitted BEFORE main(1) so the
            # scheduler prioritizes its DVE widx copies over the absdiffs
            # (they only become ready at main(1) end anyway)
            main(1)
            with tc.high_priority(offset=200):
                tail_idx(1, 0)
                tail_idx(1, 1)
            tail_gather(1)
            tail_abs(0, 0)
            tail_abs(0, 1)
            tail_abs(1, 0)
            tail_abs(1, 1)

            nc.sync.dma_start(loss_d[:], part[0:D, :])

    nc.compile()
    return nc


_NC_CACHE = None


def _get_nc():
    global _NC_CACHE
    if _NC_CACHE is None:
        _NC_CACHE = build_nc()
    return _NC_CACHE


def kernel(x: np.ndarray, gt: np.ndarray, _trace=False, _debug=False):
    x = np.ascontiguousarray(np.asarray(x, dtype=np.float32))
    gt = np.ascontiguousarray(np.asarray(gt, dtype=np.float32))
    consts = make_consts()
    nc = build_nc(debug=True) if _debug else _get_nc()
    in_maps = []
    for c in range(NCORES):
        m = {"x": x[c * B_LOC:(c + 1) * B_LOC],
             "gt": gt[c * B_LOC:(c + 1) * B_LOC]}
        m.update(consts)
        in_maps.append(m)
    res = run_bass_kernel_spmd(
        nc, in_maps, core_ids=list(range(NCORES)), trace=_trace,
        trace_cores=[0] if _trace else None,
    )
    total = sum(float(r["loss"].sum()) for r in res.results)
    out = np.asarray(np.float32(total / (B_FULL * NI * D)))
    if _trace or _debug:
        return out, res
    return out



# revision 25
# speedup vs baseline: 1.0140x; 1.0140x over previous
"""BestBuddyLoss Trainium2 kernel v2 (8-core data parallel).

Per image: q = [unfold(gt) | unfold(down2(gt)) | unfold(down4(gt))] padded to
3072 cols; argmin_j score(i,j) == argmax_j <p1_i+p2_i, q_j> - |q_j|^2, done as
K=33 f32r matmuls (27 data rows + 5 zero + bias row 32).  Logical column
j = 2k + b: even columns (b=0) stream through stride-2 matmuls into PSUM
(3 banks flat, [128,1536]); odd columns land in transit PSUM banks and are
copied to SBUF by ACT.  One 2-stream custom DVE op per i-tile scans
(A_k, B_k) pairs and emits argmax j = 2*Idx + (A<B) via running-max records
(record j's are monotone, so accum max = last record = argmax).
Tail: PMT permute -> wrapped i16 idx -> ap_gather q[j*] -> L1 reduce.

Engine schedule: image0 prep on DVE/ACT (head-critical), image1 x-path and
lhs build on Pool overlapping main(0); bias(1) slots between the mains.
"""

import sys

sys.path.insert(0, "/opt/trn_rl_repo")

import numpy as np

import concourse.bacc as bacc
import concourse.mybir as mybir
import concourse.tile as tile
from concourse.bass_utils import run_bass_kernel_spmd

# ---------------- problem constants (hardcoded) ----------------
B_FULL = 16
NCORES = 8
B_LOC = B_FULL // NCORES       # images per core
C, H, W = 3, 144, 144
G = 48                         # patch grid (144/3)
NI = G * G                     # 2304 query patches
D = 27                         # C*3*3
NQ = NI + (G // 2) ** 2 + (G // 4) ** 2  # 3024
NQP = 3072                     # padded j space (6*512)
KD = 33                        # contraction: 27 data + 5 pad + bias row 32
KZ = 32                        # bias row partition (32-aligned)
IT = 128
NIT = NI // IT                 # 18
HK = NQP // 2                  # 1536 pair count per tile
CH = 512
NCH = HK // CH                 # 3 chunks per half
PADBIAS = -1.0e30
CUBIC_W = np.array([-0.09375, 0.59375, 0.59375, -0.09375], dtype=np.float32)

F32 = mybir.dt.float32
F32R = mybir.dt.float32r
I16 = mybir.dt.int16
ADD = mybir.AluOpType.add
SUB = mybir.AluOpType.subtract
MUL = mybir.AluOpType.mult
ABS = mybir.ActivationFunctionType.Abs
SQ = mybir.ActivationFunctionType.Square

# ---------------- custom DVE op: 2-stream pair argmax ------------------
from concourse.dve_spec import Spec, Src0, Src1, C2, Zero, scan, AluOp, maxx, lower
from concourse.dve_uop import DveOpSpec
import concourse.dve_ops as dve_ops
from concourse.dve_ops import DveOp


def _pair_argmax_ref(in0, in1, c0, c1, c2):
    m = np.maximum(in0, in1)
    run = np.maximum.accumulate(m, axis=-1)
    n = in0.shape[-1]
    j = np.float32(c2) * np.arange(n, dtype=np.float32)[None, :] + (in0 < in1)
    out = (m >= run) * j
    acc = out.reshape(out.shape[0], -1).max(axis=-1, keepdims=True)
    return out.astype(np.float32), acc.astype(np.float32)


def _register_pair_op():
    name = "ANT_PAIR_ARGMAX"
    if name in dve_ops._SUB_OPCODE_FOR_NAME:
        return next(op for op in dve_ops.OPS if op.name == name)
    m = maxx(Src0, Src1)
    two_idx = scan(AluOp.ADD, C2, init=Zero - C2)   # 0, 2, 4, ... (C2=2.0)
    j = two_idx + (Src0 < Src1)
    body = (m >= scan(AluOp.MAX, m)) * j
    spec = Spec(body=body, accum=maxx, reference=_pair_argmax_ref)
    opcode = dve_ops._CUSTOM_DVE_ROW_BASE + len(dve_ops.OPS)
    shas = {v: DveOpSpec(name=name, opcode=opcode, uops=lower(spec, ver=v),
                         rd1_en=True).sha(v) for v in ("v3", "v4")}
    op = DveOp(name, spec, subdim=False, uops_sha=shas)
    dve_ops.OPS.append(op)
    dve_ops._SUB_OPCODE_FOR_NAME[name] = opcode
    dve_ops.CUSTOM_DVE_SPECS[name] = spec
    return op


PAIR_OP = _register_pair_op()


def _absdiff_acc_ref(in0, in1, c0, c1, c2):
    out = np.abs(in0 - in1).astype(np.float32)
    acc = out.reshape(out.shape[0], -1).sum(axis=-1, keepdims=True)
    return out, acc.astype(np.float32)


def _register_abs_op():
    name = "ANT_ABSDIFF_ACC"
    if name in dve_ops._SUB_OPCODE_FOR_NAME:
        return next(op for op in dve_ops.OPS if op.name == name)
    body = maxx(Src0 - Src1, Src1 - Src0)
    spec = Spec(body=body, accum=AluOp.ADD, reference=_absdiff_acc_ref)
    opcode = dve_ops._CUSTOM_DVE_ROW_BASE + len(dve_ops.OPS)
    shas = {v: DveOpSpec(name=name, opcode=opcode, uops=lower(spec, ver=v),
                         rd1_en=True).sha(v) for v in ("v3", "v4")}
    op = DveOp(name, spec, subdim=False, uops_sha=shas)
    dve_ops.OPS.append(op)
    dve_ops._SUB_OPCODE_FOR_NAME[name] = opcode
    dve_ops.CUSTOM_DVE_SPECS[name] = spec
    return op


ABS_OP = _register_abs_op()

# ---------------- host-side constants ---------------------------------


def _down_matrix(n, f):
    """M[h, i]: out[i] = sum_h M[h, i] * in[h]  (torch bicubic, offset t=.5)."""
    out_n = n // f
    M = np.zeros((n, out_n), dtype=np.float32)
    for i in range(out_n):
        base = f * i + (f // 2 - 1)
        for a in range(4):
            h = min(max(base + a - 1, 0), n - 1)
            M[h, i] += CUBIC_W[a]
    return M


def _perm_matrices():
    """PMT[:, m*128 + r]: one-hot at row (m*16 + r%16) -> out_m = Pm @ v."""
    P = np.zeros((128, 8 * 128), dtype=np.float32)
    for m in range(8):
        for r in range(128):
            P[m * 16 + r % 16, m * 128 + r] = 1.0
    return P


def make_consts():
    padq = np.zeros((KD - D, NQP), dtype=np.float32)
    padq[KZ - D, NQ:NQP] = PADBIAS
    padl = np.zeros((KD - D, NI), dtype=np.float32)
    padl[KZ - D, :] = 1.0
    return {
        "cd2": np.ascontiguousarray(_down_matrix(H, 2)),  # [144, 72]
        "cd4": np.ascontiguousarray(_down_matrix(H, 4)),  # [144, 36]
        "idn": np.eye(128, dtype=np.float32),
        "pmt": _perm_matrices(),
        "neg1": np.full((D, 1), -1.0, dtype=np.float32),
        "ones128": np.ones((128, 1), dtype=np.float32),
        "padq": padq,
        "padl": padl,
    }


# ---------------- kernel construction ---------------------------------


def build_nc(debug=False):
    nc = bacc.Bacc("TRN2", target_bir_lowering=False)

    x_d = nc.dram_tensor("x", [B_LOC, C, H, W], F32, kind="ExternalInput")
    gt_d = nc.dram_tensor("gt", [B_LOC, C, H, W], F32, kind="ExternalInput")
    cd2_d = nc.dram_tensor("cd2", [H, 72], F32, kind="ExternalInput")
    cd4_d = nc.dram_tensor("cd4", [H, 36], F32, kind="ExternalInput")
    idn_d = nc.dram_tensor("idn", [128, 128], F32, kind="ExternalInput")
    pmt_d = nc.dram_tensor("pmt", [128, 8 * 128], F32, kind="ExternalInput")
    neg1_d = nc.dram_tensor("neg1", [D, 1], F32, kind="ExternalInput")
    ones_d = nc.dram_tensor("ones128", [128, 1], F32, kind="ExternalInput")
    padq_d = nc.dram_tensor("padq", [KD - D, NQP], F32R, kind="ExternalInput")
    padl_d = nc.dram_tensor("padl", [KD - D, NI], F32R, kind="ExternalInput")
    d2_d = nc.dram_tensor("scr_d2", [B_LOC, C, 72, 72], F32, kind="Internal")
    d4_d = nc.dram_tensor("scr_d4", [B_LOC, C, 36, 36], F32, kind="Internal")
    loss_d = nc.dram_tensor("loss", [D, 2 * B_LOC], F32,
                            kind="ExternalOutput")
    dbg = {}
    if debug:
        dbg["rr"] = nc.dram_tensor("dbg_rr", [B_LOC, KD, NQP], F32, kind="ExternalOutput")
        dbg["lr"] = nc.dram_tensor("dbg_lr", [B_LOC, KD, NI], F32, kind="ExternalOutput")
        dbg["p1t"] = nc.dram_tensor("dbg_p1t", [B_LOC, D, NI], F32, kind="ExternalOutput")
        dbg["idxf"] = nc.dram_tensor("dbg_idxf", [B_LOC, 128, NIT], F32, kind="ExternalOutput")
        dbg["sel"] = nc.dram_tensor("dbg_sel", [B_LOC, 32, NI], F32, kind="ExternalOutput")

    with tile.TileContext(nc) as tc:
        with (
            tc.tile_pool(name="consts", bufs=1) as cpool,
            tc.tile_pool(name="stageA", bufs=2) as apool,
            tc.tile_pool(name="stageB", bufs=3) as bpool,
            tc.tile_pool(name="stageD", bufs=1) as dapool,
            tc.tile_pool(name="prep", bufs=2) as ppool,
            tc.tile_pool(name="persist", bufs=2) as spool,
            tc.tile_pool(name="scoreB", bufs=3) as scpool,
            tc.tile_pool(name="small", bufs=2) as smpool,
            tc.tile_pool(name="psA", bufs=2, space="PSUM") as psA,
            tc.tile_pool(name="psB", bufs=2, space="PSUM") as psB,
        ):
            cd2a_f = cpool.tile([128, 72], F32, tag="cd2af")
            cd2b_f = cpool.tile([16, 72], F32, tag="cd2bf")
            cd4a_f = cpool.tile([128, 36], F32, tag="cd4af")
            cd4b_f = cpool.tile([16, 36], F32, tag="cd4bf")
            cd2a = cpool.tile([128, 72], F32R, tag="cd2a")
            cd2b = cpool.tile([16, 72], F32R, tag="cd2b")
            cd4a = cpool.tile([128, 36], F32R, tag="cd4a")
            cd4b = cpool.tile([16, 36], F32R, tag="cd4b")
            idn_t = cpool.tile([128, 128], F32, tag="idn")
            pmt_t = cpool.tile([128, 8 * 128], F32, tag="pmt")
            neg1_t = cpool.tile([D, 1], F32, tag="neg1")
            neg1r = cpool.tile([D, 1], F32R, tag="neg1r")
            ones_t = cpool.tile([128, 1], F32, tag="ones")

            junk = cpool.tile([128, HK], F32, tag="junk")
            part = cpool.tile([128, 2 * B_LOC], F32, tag="part")

            def shuffle(eng, dst_v, src_v):
                if eng is nc.scalar:
                    eng.copy(dst_v, src_v)
                else:
                    eng.tensor_copy(dst_v, src_v)

            def unfold_big(dram_plane_ap, dst_ap, eng, round_f32r, nm):
                """dram [C, 144, 144] -> dst [27, 2304]; at loads on the SP
                queue (ready at t=0), shuffle + out-DMA on the shuffling
                engine's queue (no SP head-of-line blocking)."""
                g, gh = G, G // 2
                szh = 3 * gh * g
                for hf in range(2):
                    at = apool.tile([9, szh], F32, tag="A", name=f"at_{nm}{hf}")
                    for c in range(C):
                        src = dram_plane_ap[c].rearrange(
                            "(gi r) w -> r gi w", r=3
                        )[:, hf * gh:(hf + 1) * gh, :]
                        nc.sync.dma_start(at[3 * c:3 * c + 3, :], src)
                    bt = bpool.tile([9, szh], F32R if round_f32r else F32,
                                    tag="B", name=f"bt_{nm}{hf}")
                    rearr = at[:, :].rearrange(
                        "p (gi gj s) -> p s gi gj", gi=gh, gj=g, s=3)
                    bt_v = bt[:, :].rearrange(
                        "p (s gi gj) -> p s gi gj", s=3, gi=gh)
                    shuffle(eng, bt_v, rearr)
                    dma_eng = nc.gpsimd if eng is nc.gpsimd else nc.scalar
                    fh = gh * g
                    dst_half = dst_ap[:, hf * fh:(hf + 1) * fh]
                    if round_f32r:
                        dma_eng.dma_start(dst_half.bitcast(F32),
                                          bt[:, :].bitcast(F32))
                    else:
                        dma_eng.dma_start(dst_half, bt[:, :])

            def unfold_small(dsc_d, f, dst_ap, eng, nm):
                """downsampled DRAM scratch [C, n, n] ->
                dst [27, gsub*gsub]."""
                n = H // f
                gsub = n // 3
                sz = 3 * gsub * gsub
                dat = dapool.tile([9, 3 * 24 * 24], F32, tag="DA",
                                  name=f"dat_{nm}")
                dma_eng = nc.gpsimd if eng is nc.gpsimd else nc.scalar
                for c in range(C):
                    src = dsc_d[c].rearrange("(gi r) w -> r gi w", r=3)
                    dma_eng.dma_start(
                        dat[3 * c:3 * c + 3, 0:gsub * n].rearrange(
                            "p (gi w) -> p gi w", gi=gsub), src)
                bt = bpool.tile([9, 3 * (G // 2) * G], F32R, tag="B",
                                name=f"bt_{nm}")
                rearr = dat[:, 0:sz].rearrange(
                    "p (gi gj s) -> p s gi gj", gi=gsub, gj=gsub, s=3)
                bt_v = bt[:, 0:sz].rearrange(
                    "p (s gi gj) -> p s gi gj", s=3, gi=gsub)
                shuffle(eng, bt_v, rearr)
                dma_eng.dma_start(dst_ap.bitcast(F32),
                                  bt[:, 0:sz].bitcast(F32))

            def downsample(b, f, ga, gb, out_dram, psp=None, pstag="psB",
                           ceng=None):
                """gt[b] --bicubic/f--> out_dram [C, n, n]."""
                psp = psp or psB
                def ptile():
                    t = psp.tile([128, NCH, CH] if psp is psA else [128, CH],
                                 F32, tag=pstag)
                    return t[:, 0, :] if psp is psA else t
                n = H // f
                cda = cd2a if f == 2 else cd4a
                cdb = cd2b if f == 2 else cd4b
                cdaf = cd2a_f if f == 2 else cd4a_f
                cdbf = cd2b_f if f == 2 else cd4b_f
                ghp = ptile()
                nc.tensor.matmul(ghp[0:n, 0:C * H], cda[:, 0:n], ga[:],
                                 start=True, stop=False)
                nc.tensor.matmul(ghp[0:n, 0:C * H], cdb[:, 0:n], gb[:],
                                 start=False, stop=True)
                ceng = ceng or nc.scalar
                def pcopy(dst, srcp):
                    if ceng is nc.scalar:
                        ceng.copy(dst, srcp)
                    else:
                        ceng.tensor_copy(dst, srcp)
                gh = ppool.tile([72, C * H], F32, tag="gh", bufs=1)
                pcopy(gh[0:n, :], ghp[0:n, 0:C * H])
                gh3 = gh[:].rearrange("i (c w) -> i c w", c=C)
                ghta = ppool.tile([128, C * 72], F32R, tag="ghta")
                ghtb = ppool.tile([16, C * 72], F32R, tag="ghtb")
                ghta3 = ghta[:].rearrange("w (c i) -> w c i", c=C)
                ghtb3 = ghtb[:].rearrange("w (c i) -> w c i", c=C)
                for c in range(C):
                    tp = ptile()
                    nc.tensor.transpose(tp[0:128, 0:n], gh3[0:n, c, 0:128],
                                        idn_t[0:n, 0:n])
                    pcopy(ghta3[:, c, 0:n], tp[0:128, 0:n])
                    tp2 = ptile()
                    nc.tensor.transpose(tp2[0:16, 0:n],
                                        gh3[0:n, c, 128:144], idn_t[0:n, 0:n])
                    pcopy(ghtb3[:, c, 0:n], tp2[0:16, 0:n])
                g2 = ppool.tile([72, C * 72], F32, tag=f"g2_{f}")
                g23 = g2[:].rearrange("i (c j) -> i c j", c=C)
                for c in range(C):
                    op = ptile()
                    nc.tensor.matmul(op[0:n, 0:n], ghta3[:, c, 0:n],
                                     cda[:, 0:n], start=True, stop=False)
                    nc.tensor.matmul(op[0:n, 0:n], ghtb3[:, c, 0:n],
                                     cdb[:, 0:n], start=False, stop=True)
                    pcopy(g23[0:n, c, 0:n], op[0:n, 0:n])
                out_ap = out_dram.rearrange("c h w -> h c w")
                nc.scalar.dma_start(out_ap, g23[0:n, :, 0:n])

            rrs = [None] * B_LOC
            lrs = [None] * B_LOC
            p1ts = [None] * B_LOC
            idxf = [None] * B_LOC
            widxs = [None] * B_LOC
            sels = [None] * B_LOC

            def prep_head(b, eng):
                """pads, downsamples, gt/d2/d4 unfolds -> rr q rows."""
                rr = spool.tile([KD, NQP], F32R, tag="rr")
                lr = spool.tile([KD, NI], F32R, tag="lr")
                p1t = spool.tile([D, NI], F32, tag="p1t")
                rrs[b], lrs[b], p1ts[b] = rr, lr, p1t
                idxf[b] = smpool.tile([128, NIT], F32, tag="idxf",
                                      name=f"idxf{b}")
                widxs[b] = smpool.tile([128, 8 * NIT], I16, tag="widx",
                                       name=f"widx{b}")
                sels[b] = smpool.tile([32, NI], F32, tag="sel", bufs=1,
                                      name=f"sel{b}")
                # pad rows 27:33 (zeros + bias/ones rows) via one DMA each
                nc.gpsimd.dma_start(rr[D:KD, :], padq_d[:])
                nc.gpsimd.dma_start(lr[D:KD, :], padl_d[:])
                nc.gpsimd.memset(rr[0:KZ, NQ:NQP].bitcast(F32), 0.0)

                ga = ppool.tile([128, C * H], F32, tag="gplane_a", bufs=1)
                gb = ppool.tile([16, C * H], F32, tag="gplane_b", bufs=1)
                gar = ppool.tile([128, C * H], F32R, tag="gplane_ar", bufs=1)
                gbr = ppool.tile([16, C * H], F32R, tag="gplane_br", bufs=1)
                gsrc = gt_d[b].rearrange("c h w -> h c w")
                nc.sync.dma_start(ga[:], gsrc[0:128])
                nc.sync.dma_start(gb[:], gsrc[128:144])
                ceng = nc.vector if b == 0 else nc.gpsimd
                ceng.tensor_copy(gar[:], ga[:])
                ceng.tensor_copy(gbr[:], gb[:])
                psp = psA if b == 0 else psB
                downsample(b, 4, gar, gbr, d4_d[b], psp=psp,
                           pstag="psA" if b == 0 else "psB",
                           ceng=None if b == 0 else nc.vector)
                downsample(b, 2, gar, gbr, d2_d[b], psp=psp,
                           pstag="psA" if b == 0 else "psB",
                           ceng=None if b == 0 else nc.vector)
                unfold_big(gt_d[b], rr[0:D, 0:NI], eng, True, f"gt{b}")

            def prep_dsmall(b, eng):
                rr = rrs[b]
                unfold_small(d4_d[b], 4, rr[0:D, NI + 576:NQ], eng, f"d4{b}")
                unfold_small(d2_d[b], 2, rr[0:D, NI:NI + 576], eng, f"d2{b}")

            qsqs = [None] * B_LOC

            def get_qsq(b):
                if qsqs[b] is None:
                    qsqs[b] = ppool.tile([D, NQ], F32R, tag="qsq", bufs=1,
                                         name=f"qsq{b}")
                return qsqs[b]

            def qsq_sq(b, part_, use_pool):
                rr = rrs[b]
                qsq = get_qsq(b)
                lo, hi = (0, 2016) if part_ == 0 else (2016, NQ)
                if use_pool == "pool":
                    nc.gpsimd.tensor_tensor(qsq[:, lo:hi],
                                            rr[0:D, lo:hi].bitcast(F32),
                                            rr[0:D, lo:hi].bitcast(F32),
                                            op=MUL)
                elif use_pool == "dve":
                    nc.vector.tensor_tensor(qsq[:, lo:hi],
                                            rr[0:D, lo:hi].bitcast(F32),
                                            rr[0:D, lo:hi].bitcast(F32),
                                            op=MUL)
                else:
                    nc.scalar.activation(qsq[:, lo:hi],
                                         rr[0:D, lo:hi].bitcast(F32), SQ)

            def qsq_bias_row(b, part_, psp=None, pstag="psB", split=False):
                psp = psp or psB
                rr = rrs[b]
                qsq = get_qsq(b)
                lo, hi = (0, 2016) if part_ == 0 else (2016, NQ)
                for jt in range(lo // 504, hi // 504):
                    if psp is psA:
                        pt = psp.tile([128, NCH, CH], F32, tag=pstag)
                        bnp = pt[:, 0, :]
                    else:
                        bnp = psp.tile([128, CH], F32, tag=pstag)
                    nc.tensor.matmul(bnp[0:1, 0:504], neg1r[:],
                                     qsq[:, jt * 504:(jt + 1) * 504])
                    sl = rr[KZ:KZ + 1, jt * 504:(jt + 1) * 504]
                    if split and jt % 2 == 1:
                        nc.vector.tensor_copy(sl, bnp[0:1, 0:504])
                    else:
                        nc.scalar.copy(sl, bnp[0:1, 0:504])
                if debug and part_ == 1:
                    nc.sync.dma_start(dbg["rr"][b], rr[:, :].bitcast(F32))

            def qsq_bias(b, use_pool, part_, psp=None, pstag="psB"):
                qsq_sq(b, part_, use_pool)
                qsq_bias_row(b, part_, psp=psp, pstag=pstag)

            def prep_x(b, eng_shuf, eng_add):
                """p1t = unfold(x); lr rows 0:27 = p1 + p2."""
                unfold_big(x_d[b], p1ts[b][:, :], eng_shuf, False, f"x{b}")
                eng_add.tensor_tensor(lrs[b][0:D, :], p1ts[b][:, :],
                                      rrs[b][0:D, 0:NI].bitcast(F32), op=ADD)
                if debug:
                    nc.sync.dma_start(dbg["lr"][b], lrs[b][:, :].bitcast(F32))
                    nc.sync.dma_start(dbg["p1t"][b], p1ts[b][:, :])

            def main(b, hook=None):
                lr, rr = lrs[b], rrs[b]
                rv = rr[:, :].rearrange("p (k two) -> p k two", two=2)
                for t in range(NIT):
                    if hook is not None:
                        hook(t)
                    lrt = lr[:, t * IT:(t + 1) * IT]
                    scB = scpool.tile([128, HK], F32, tag="scB")
                    for c in range(NCH):
                        pb = psB.tile([128, CH], F32, tag="psB")
                        nc.tensor.matmul(pb[:, :], lrt,
                                         rv[:, CH * c:CH * (c + 1), 1])
                        nc.scalar.copy(scB[:, CH * c:CH * (c + 1)],
                                       pb[:, :])
                    pa = psA.tile([128, NCH, CH], F32, tag="psA")
                    for c in range(NCH):
                        nc.tensor.matmul(pa[:, c, :], lrt,
                                         rv[:, CH * c:CH * (c + 1), 0])
                    flatA = pa[:, :, :].rearrange("p a b -> p (a b)")
                    nc.vector._custom_dve(
                        PAIR_OP, out=junk[:], in0=flatA[:, :], in1=scB[:, :],
                        accum_out=idxf[b][:, t:t + 1], imm2=2.0,
                    )
                if debug:
                    nc.sync.dma_start(dbg["idxf"][b], idxf[b][:, :])

            HT = NIT // 2
            HNI = NI // 2

            def tail_idx(b, h):
                """PMT permute + wrapped i16 idx for tiles [h*9,(h+1)*9)."""
                wp = psB.tile([128, CH], F32, tag="psB",
                              name=f"pswp{b}_{h}")
                wp3 = wp[0:128, 0:8 * HT].rearrange("p (m t) -> p m t", m=8)
                for m in range(8):
                    nc.tensor.matmul(
                        wp3[:, m, :], pmt_t[:, m * 128:(m + 1) * 128],
                        idxf[b][:, h * HT:(h + 1) * HT],
                    )
                w3 = widxs[b][:].rearrange("p (t m) -> p t m", t=NIT)
                nc.vector.tensor_copy(
                    w3[:, h * HT:(h + 1) * HT, :],
                    wp3[:, :, :].rearrange("p m t -> p t m"))

            def tail_gather(b):
                sel = sels[b]
                nc.gpsimd.ap_gather(
                    sel[:, :].rearrange("p (n d) -> p n d", d=1),
                    rrs[b][0:32, :].bitcast(F32).rearrange(
                        "p (n d) -> p n d", d=1),
                    widxs[b][0:32, :],
                    channels=32, num_elems=NQP, d=1, num_idxs=NI,
                )
                if debug:
                    nc.sync.dma_start(dbg["sel"][b], sel[:, :])
                    nc.sync.dma_start(dbg["idxf"][b], idxf[b][:, :])

            def tail_abs(b, h):
                nc.vector._custom_dve(
                    ABS_OP, out=junk[0:D, 0:HNI],
                    in0=sels[b][0:D, h * HNI:(h + 1) * HNI],
                    in1=p1ts[b][:, h * HNI:(h + 1) * HNI],
                    accum_out=part[0:D, 2 * b + h:2 * b + h + 1],
                    imm2=0.0,
                )

            # ---------------- orchestration ----------------
            # const loads on the ACT hwdge queue so image-0's at-loads own
            # the first SP-queue HWDGE slots
            nc.scalar.dma_start(cd4a_f[:], cd4_d[0:128, :])
            nc.scalar.dma_start(cd4b_f[:], cd4_d[128:144, :])
            nc.scalar.dma_start(cd2a_f[:], cd2_d[0:128, :])
            nc.scalar.dma_start(cd2b_f[:], cd2_d[128:144, :])
            nc.scalar.dma_start(idn_t[:], idn_d[:])
            nc.scalar.dma_start(neg1_t[:], neg1_d[:])
            nc.scalar.dma_start(ones_t[:], ones_d[:])
            nc.vector.tensor_copy(cd4a[:], cd4a_f[:])
            nc.vector.tensor_copy(cd4b[:], cd4b_f[:])
            nc.vector.tensor_copy(cd2a[:], cd2a_f[:])
            nc.vector.tensor_copy(cd2b[:], cd2b_f[:])
            nc.vector.tensor_copy(neg1r[:], neg1_t[:])
            prep_head(0, nc.vector)    # image0 on DVE (head-critical)
            nc.scalar.dma_start(pmt_t[:], pmt_d[:])
            prep_x(0, nc.vector, nc.vector)
            prep_dsmall(0, nc.vector)
            qsq_bias(0, use_pool="act", part_=0, psp=psA, pstag="psA")
            qsq_bias(0, use_pool="act", part_=1, psp=psA, pstag="psA")
            prep_head(1, nc.gpsimd)    # image1 on Pool; ds copies on DVE
            prep_x(1, nc.gpsimd, nc.gpsimd)
            prep_dsmall(1, nc.gpsimd)

            main(0)
            qsq_sq(1, 0, use_pool="dve")
            qsq_bias_row(1, 0, split=True)
            qsq_sq(1, 1, use_pool="dve")
            qsq_bias_row(1, 1, split=True)
            # image-0 tail (gather overlaps main(1) on Pool)
            tail_idx(0, 0)
            tail_idx(0, 1)
            tail_gather(0)
            # image-1 idx/gather chain emitted BEFORE main(1) so the
            # scheduler prioritizes its DVE widx copies over the absdiffs
            # (they only become ready at main(1) end anyway)
            main(1)
            with tc.high_priority(offset=200):
                tail_idx(1, 0)
                tail_idx(1, 1)
            tail_gather(1)
            tail_abs(0, 0)
            tail_abs(0, 1)
            tail_abs(1, 0)
            tail_abs(1, 1)

            nc.sync.dma_start(loss_d[:], part[0:D, :])

    nc.compile()
    return nc


_NC_CACHE = None


def _get_nc():
    global _NC_CACHE
    if _NC_CACHE is None:
        _NC_CACHE = build_nc()
    return _NC_CACHE


def kernel(x: np.ndarray, gt: np.ndarray, _trace=False, _debug=False):
    x = np.ascontiguousarray(np.asarray(x, dtype=np.float32))
    gt = np.ascontiguousarray(np.asarray(gt, dtype=np.float32))
    consts = make_consts()
    nc = build_nc(debug=True) if _debug else _get_nc()
    in_maps = []
    for c in range(NCORES):
        m = {"x": x[c * B_LOC:(c + 1) * B_LOC],
             "gt": gt[c * B_LOC:(c + 1) * B_LOC]}
        m.update(consts)
        in_maps.append(m)
    res = run_bass_kernel_spmd(
        nc, in_maps, core_ids=list(range(NCORES)), trace=_trace,
        trace_cores=[0] if _trace else None,
    )
    total = sum(float(r["loss"].sum()) for r in res.results)
    out = np.asarray(np.float32(total / (B_FULL * NI * D)))
    if _trace or _debug:
        return out, res
    return out



# revision 26
# speedup vs baseline: 1.0225x; 1.0084x over previous
"""BestBuddyLoss Trainium2 kernel v2 (8-core data parallel).

Per image: q = [unfold(gt) | unfold(down2(gt)) | unfold(down4(gt))] padded to
3072 cols; argmin_j score(i,j) == argmax_j <p1_i+p2_i, q_j> - |q_j|^2, done as
K=33 f32r matmuls (27 data rows + 5 zero + bias row 32).  Logical column
j = 2k + b: even columns (b=0) stream through stride-2 matmuls into PSUM
(3 banks flat, [128,1536]); odd columns land in transit PSUM banks and are
copied to SBUF by ACT.  One 2-stream custom DVE op per i-tile scans
(A_k, B_k) pairs and emits argmax j = 2*Idx + (A<B) via running-max records
(record j's are monotone, so accum max = last record = argmax).
Tail: PMT permute -> wrapped i16 idx -> ap_gather q[j*] -> L1 reduce.

Engine schedule: image0 prep on DVE/ACT (head-critical), image1 x-path and
lhs build on Pool overlapping main(0); bias(1) slots between the mains.
"""

import sys

sys.path.insert(0, "/opt/trn_rl_repo")

import numpy as np

import concourse.bacc as bacc
import concourse.mybir as mybir
import concourse.tile as tile
from concourse.bass_utils import run_bass_kernel_spmd

# ---------------- problem constants (hardcoded) ----------------
B_FULL = 16
NCORES = 8
B_LOC = B_FULL // NCORES       # images per core
C, H, W = 3, 144, 144
G = 48                         # patch grid (144/3)
NI = G * G                     # 2304 query patches
D = 27                         # C*3*3
NQ = NI + (G // 2) ** 2 + (G // 4) ** 2  # 3024
NQP = 3072                     # padded j space (6*512)
KD = 33                        # contraction: 27 data + 5 pad + bias row 32
KZ = 32                        # bias row partition (32-aligned)
IT = 128
NIT = NI // IT                 # 18
HK = NQP // 2                  # 1536 pair count per tile
CH = 512
NCH = HK // CH                 # 3 chunks per half
PADBIAS = -1.0e30
CUBIC_W = np.array([-0.09375, 0.59375, 0.59375, -0.09375], dtype=np.float32)

F32 = mybir.dt.float32
F32R = mybir.dt.float32r
I16 = mybir.dt.int16
ADD = mybir.AluOpType.add
SUB = mybir.AluOpType.subtract
MUL = mybir.AluOpType.mult
ABS = mybir.ActivationFunctionType.Abs
SQ = mybir.ActivationFunctionType.Square

# ---------------- custom DVE op: 2-stream pair argmax ------------------
from concourse.dve_spec import Spec, Src0, Src1, C2, Zero, scan, AluOp, maxx, lower
from concourse.dve_uop import DveOpSpec
import concourse.dve_ops as dve_ops
from concourse.dve_ops import DveOp


def _pair_argmax_ref(in0, in1, c0, c1, c2):
    m = np.maximum(in0, in1)
    run = np.maximum.accumulate(m, axis=-1)
    n = in0.shape[-1]
    j = np.float32(c2) * np.arange(n, dtype=np.float32)[None, :] + (in0 < in1)
    out = (m >= run) * j
    acc = out.reshape(out.shape[0], -1).max(axis=-1, keepdims=True)
    return out.astype(np.float32), acc.astype(np.float32)


def _register_pair_op():
    name = "ANT_PAIR_ARGMAX"
    if name in dve_ops._SUB_OPCODE_FOR_NAME:
        return next(op for op in dve_ops.OPS if op.name == name)
    m = maxx(Src0, Src1)
    two_idx = scan(AluOp.ADD, C2, init=Zero - C2)   # 0, 2, 4, ... (C2=2.0)
    j = two_idx + (Src0 < Src1)
    body = (m >= scan(AluOp.MAX, m)) * j
    spec = Spec(body=body, accum=maxx, reference=_pair_argmax_ref)
    opcode = dve_ops._CUSTOM_DVE_ROW_BASE + len(dve_ops.OPS)
    shas = {v: DveOpSpec(name=name, opcode=opcode, uops=lower(spec, ver=v),
                         rd1_en=True).sha(v) for v in ("v3", "v4")}
    op = DveOp(name, spec, subdim=False, uops_sha=shas)
    dve_ops.OPS.append(op)
    dve_ops._SUB_OPCODE_FOR_NAME[name] = opcode
    dve_ops.CUSTOM_DVE_SPECS[name] = spec
    return op


PAIR_OP = _register_pair_op()


def _absdiff_acc_ref(in0, in1, c0, c1, c2):
    out = np.abs(in0 - in1).astype(np.float32)
    acc = out.reshape(out.shape[0], -1).sum(axis=-1, keepdims=True)
    return out, acc.astype(np.float32)


def _register_abs_op():
    name = "ANT_ABSDIFF_ACC"
    if name in dve_ops._SUB_OPCODE_FOR_NAME:
        return next(op for op in dve_ops.OPS if op.name == name)
    body = maxx(Src0 - Src1, Src1 - Src0)
    spec = Spec(body=body, accum=AluOp.ADD, reference=_absdiff_acc_ref)
    opcode = dve_ops._CUSTOM_DVE_ROW_BASE + len(dve_ops.OPS)
    shas = {v: DveOpSpec(name=name, opcode=opcode, uops=lower(spec, ver=v),
                         rd1_en=True).sha(v) for v in ("v3", "v4")}
    op = DveOp(name, spec, subdim=False, uops_sha=shas)
    dve_ops.OPS.append(op)
    dve_ops._SUB_OPCODE_FOR_NAME[name] = opcode
    dve_ops.CUSTOM_DVE_SPECS[name] = spec
    return op


ABS_OP = _register_abs_op()

# ---------------- host-side constants ---------------------------------


def _down_matrix(n, f):
    """M[h, i]: out[i] = sum_h M[h, i] * in[h]  (torch bicubic, offset t=.5)."""
    out_n = n // f
    M = np.zeros((n, out_n), dtype=np.float32)
    for i in range(out_n):
        base = f * i + (f // 2 - 1)
        for a in range(4):
            h = min(max(base + a - 1, 0), n - 1)
            M[h, i] += CUBIC_W[a]
    return M


def _perm_matrices():
    """PMT[:, m*128 + r]: one-hot at row (m*16 + r%16) -> out_m = Pm @ v."""
    P = np.zeros((128, 8 * 128), dtype=np.float32)
    for m in range(8):
        for r in range(128):
            P[m * 16 + r % 16, m * 128 + r] = 1.0
    return P


def make_consts():
    padq = np.zeros((KD - D, NQP), dtype=np.float32)
    padq[KZ - D, NQ:NQP] = PADBIAS
    padl = np.zeros((KD - D, NI), dtype=np.float32)
    padl[KZ - D, :] = 1.0
    return {
        "cd2": np.ascontiguousarray(_down_matrix(H, 2)),  # [144, 72]
        "cd4": np.ascontiguousarray(_down_matrix(H, 4)),  # [144, 36]
        "idn": np.eye(128, dtype=np.float32),
        "pmt": _perm_matrices(),
        "neg1": np.full((D, 1), -1.0, dtype=np.float32),
        "ones128": np.ones((128, 1), dtype=np.float32),
        "padq": padq,
        "padl": padl,
    }


# ---------------- kernel construction ---------------------------------


def build_nc(debug=False):
    nc = bacc.Bacc("TRN2", target_bir_lowering=False)

    x_d = nc.dram_tensor("x", [B_LOC, C, H, W], F32, kind="ExternalInput")
    gt_d = nc.dram_tensor("gt", [B_LOC, C, H, W], F32, kind="ExternalInput")
    cd2_d = nc.dram_tensor("cd2", [H, 72], F32, kind="ExternalInput")
    cd4_d = nc.dram_tensor("cd4", [H, 36], F32, kind="ExternalInput")
    idn_d = nc.dram_tensor("idn", [128, 128], F32, kind="ExternalInput")
    pmt_d = nc.dram_tensor("pmt", [128, 8 * 128], F32, kind="ExternalInput")
    neg1_d = nc.dram_tensor("neg1", [D, 1], F32, kind="ExternalInput")
    ones_d = nc.dram_tensor("ones128", [128, 1], F32, kind="ExternalInput")
    padq_d = nc.dram_tensor("padq", [KD - D, NQP], F32R, kind="ExternalInput")
    padl_d = nc.dram_tensor("padl", [KD - D, NI], F32R, kind="ExternalInput")
    d2_d = nc.dram_tensor("scr_d2", [B_LOC, C, 72, 72], F32, kind="Internal")
    d4_d = nc.dram_tensor("scr_d4", [B_LOC, C, 36, 36], F32, kind="Internal")
    loss_d = nc.dram_tensor("loss", [D, 2 * B_LOC], F32,
                            kind="ExternalOutput")
    dbg = {}
    if debug:
        dbg["rr"] = nc.dram_tensor("dbg_rr", [B_LOC, KD, NQP], F32, kind="ExternalOutput")
        dbg["lr"] = nc.dram_tensor("dbg_lr", [B_LOC, KD, NI], F32, kind="ExternalOutput")
        dbg["p1t"] = nc.dram_tensor("dbg_p1t", [B_LOC, D, NI], F32, kind="ExternalOutput")
        dbg["idxf"] = nc.dram_tensor("dbg_idxf", [B_LOC, 128, NIT], F32, kind="ExternalOutput")
        dbg["sel"] = nc.dram_tensor("dbg_sel", [B_LOC, 32, NI], F32, kind="ExternalOutput")

    with tile.TileContext(nc) as tc:
        with (
            tc.tile_pool(name="consts", bufs=1) as cpool,
            tc.tile_pool(name="stageA", bufs=2) as apool,
            tc.tile_pool(name="stageB", bufs=3) as bpool,
            tc.tile_pool(name="stageD", bufs=1) as dapool,
            tc.tile_pool(name="prep", bufs=2) as ppool,
            tc.tile_pool(name="persist", bufs=2) as spool,
            tc.tile_pool(name="scoreB", bufs=3) as scpool,
            tc.tile_pool(name="small", bufs=2) as smpool,
            tc.tile_pool(name="psA", bufs=2, space="PSUM") as psA,
            tc.tile_pool(name="psB", bufs=2, space="PSUM") as psB,
        ):
            cd2a_f = cpool.tile([128, 72], F32, tag="cd2af")
            cd2b_f = cpool.tile([16, 72], F32, tag="cd2bf")
            cd4a_f = cpool.tile([128, 36], F32, tag="cd4af")
            cd4b_f = cpool.tile([16, 36], F32, tag="cd4bf")
            cd2a = cpool.tile([128, 72], F32R, tag="cd2a")
            cd2b = cpool.tile([16, 72], F32R, tag="cd2b")
            cd4a = cpool.tile([128, 36], F32R, tag="cd4a")
            cd4b = cpool.tile([16, 36], F32R, tag="cd4b")
            idn_t = cpool.tile([128, 128], F32, tag="idn")
            pmt_t = cpool.tile([128, 8 * 128], F32, tag="pmt")
            neg1_t = cpool.tile([D, 1], F32, tag="neg1")
            neg1r = cpool.tile([D, 1], F32R, tag="neg1r")
            ones_t = cpool.tile([128, 1], F32, tag="ones")

            junk = cpool.tile([128, HK], F32, tag="junk")
            part = cpool.tile([128, 2 * B_LOC], F32, tag="part")

            def shuffle(eng, dst_v, src_v):
                if eng is nc.scalar:
                    eng.copy(dst_v, src_v)
                else:
                    eng.tensor_copy(dst_v, src_v)

            def unfold_big(dram_plane_ap, dst_ap, eng, round_f32r, nm):
                """dram [C, 144, 144] -> dst [27, 2304]; at loads on the SP
                queue (ready at t=0), shuffle + out-DMA on the shuffling
                engine's queue (no SP head-of-line blocking)."""
                g, gh = G, G // 2
                szh = 3 * gh * g
                for hf in range(2):
                    at = apool.tile([9, szh], F32, tag="A", name=f"at_{nm}{hf}")
                    for c in range(C):
                        src = dram_plane_ap[c].rearrange(
                            "(gi r) w -> r gi w", r=3
                        )[:, hf * gh:(hf + 1) * gh, :]
                        nc.sync.dma_start(at[3 * c:3 * c + 3, :], src)
                    bt = bpool.tile([9, szh], F32R if round_f32r else F32,
                                    tag="B", name=f"bt_{nm}{hf}")
                    rearr = at[:, :].rearrange(
                        "p (gi gj s) -> p s gi gj", gi=gh, gj=g, s=3)
                    bt_v = bt[:, :].rearrange(
                        "p (s gi gj) -> p s gi gj", s=3, gi=gh)
                    shuffle(eng, bt_v, rearr)
                    dma_eng = nc.gpsimd if eng is nc.gpsimd else nc.scalar
                    fh = gh * g
                    dst_half = dst_ap[:, hf * fh:(hf + 1) * fh]
                    if round_f32r:
                        dma_eng.dma_start(dst_half.bitcast(F32),
                                          bt[:, :].bitcast(F32))
                    else:
                        dma_eng.dma_start(dst_half, bt[:, :])

            def unfold_small(dsc_d, f, dst_ap, eng, nm):
                """downsampled DRAM scratch [C, n, n] ->
                dst [27, gsub*gsub]."""
                n = H // f
                gsub = n // 3
                sz = 3 * gsub * gsub
                dat = dapool.tile([9, 3 * 24 * 24], F32, tag="DA",
                                  name=f"dat_{nm}")
                dma_eng = nc.gpsimd if eng is nc.gpsimd else nc.scalar
                for c in range(C):
                    src = dsc_d[c].rearrange("(gi r) w -> r gi w", r=3)
                    dma_eng.dma_start(
                        dat[3 * c:3 * c + 3, 0:gsub * n].rearrange(
                            "p (gi w) -> p gi w", gi=gsub), src)
                bt = bpool.tile([9, 3 * (G // 2) * G], F32R, tag="B",
                                name=f"bt_{nm}")
                rearr = dat[:, 0:sz].rearrange(
                    "p (gi gj s) -> p s gi gj", gi=gsub, gj=gsub, s=3)
                bt_v = bt[:, 0:sz].rearrange(
                    "p (s gi gj) -> p s gi gj", s=3, gi=gsub)
                shuffle(eng, bt_v, rearr)
                dma_eng.dma_start(dst_ap.bitcast(F32),
                                  bt[:, 0:sz].bitcast(F32))

            def downsample(b, f, ga, gb, out_dram, psp=None, pstag="psB",
                           ceng=None):
                """gt[b] --bicubic/f--> out_dram [C, n, n]."""
                psp = psp or psB
                def ptile():
                    t = psp.tile([128, NCH, CH] if psp is psA else [128, CH],
                                 F32, tag=pstag)
                    return t[:, 0, :] if psp is psA else t
                n = H // f
                cda = cd2a if f == 2 else cd4a
                cdb = cd2b if f == 2 else cd4b
                cdaf = cd2a_f if f == 2 else cd4a_f
                cdbf = cd2b_f if f == 2 else cd4b_f
                ghp = ptile()
                nc.tensor.matmul(ghp[0:n, 0:C * H], cda[:, 0:n], ga[:],
                                 start=True, stop=False)
                nc.tensor.matmul(ghp[0:n, 0:C * H], cdb[:, 0:n], gb[:],
                                 start=False, stop=True)
                ceng = ceng or nc.scalar
                def pcopy(dst, srcp):
                    if ceng is nc.scalar:
                        ceng.copy(dst, srcp)
                    else:
                        ceng.tensor_copy(dst, srcp)
                gh = ppool.tile([72, C * H], F32, tag="gh", bufs=1)
                pcopy(gh[0:n, :], ghp[0:n, 0:C * H])
                gh3 = gh[:].rearrange("i (c w) -> i c w", c=C)
                ghta = ppool.tile([128, C * 72], F32R, tag="ghta")
                ghtb = ppool.tile([16, C * 72], F32R, tag="ghtb")
                ghta3 = ghta[:].rearrange("w (c i) -> w c i", c=C)
                ghtb3 = ghtb[:].rearrange("w (c i) -> w c i", c=C)
                for c in range(C):
                    tp = ptile()
                    nc.tensor.transpose(tp[0:128, 0:n], gh3[0:n, c, 0:128],
                                        idn_t[0:n, 0:n])
                    pcopy(ghta3[:, c, 0:n], tp[0:128, 0:n])
                    tp2 = ptile()
                    nc.tensor.transpose(tp2[0:16, 0:n],
                                        gh3[0:n, c, 128:144], idn_t[0:n, 0:n])
                    pcopy(ghtb3[:, c, 0:n], tp2[0:16, 0:n])
                g2 = ppool.tile([72, C * 72], F32, tag=f"g2_{f}")
                g23 = g2[:].rearrange("i (c j) -> i c j", c=C)
                for c in range(C):
                    op = ptile()
                    nc.tensor.matmul(op[0:n, 0:n], ghta3[:, c, 0:n],
                                     cda[:, 0:n], start=True, stop=False)
                    nc.tensor.matmul(op[0:n, 0:n], ghtb3[:, c, 0:n],
                                     cdb[:, 0:n], start=False, stop=True)
                    pcopy(g23[0:n, c, 0:n], op[0:n, 0:n])
                out_ap = out_dram.rearrange("c h w -> h c w")
                nc.scalar.dma_start(out_ap, g23[0:n, :, 0:n])

            rrs = [None] * B_LOC
            lrs = [None] * B_LOC
            p1ts = [None] * B_LOC
            idxf = [None] * B_LOC
            widxs = [None] * B_LOC
            sels = [None] * B_LOC

            def prep_head(b, eng):
                """pads, downsamples, gt/d2/d4 unfolds -> rr q rows."""
                rr = spool.tile([KD, NQP], F32R, tag="rr")
                lr = spool.tile([KD, NI], F32R, tag="lr")
                p1t = spool.tile([D, NI], F32, tag="p1t")
                rrs[b], lrs[b], p1ts[b] = rr, lr, p1t
                idxf[b] = smpool.tile([128, NIT], F32, tag="idxf",
                                      name=f"idxf{b}")
                widxs[b] = smpool.tile([128, 8 * NIT], I16, tag="widx",
                                       name=f"widx{b}")
                sels[b] = smpool.tile([32, NI], F32, tag="sel", bufs=1,
                                      name=f"sel{b}")
                # pad rows 27:33 (zeros + bias/ones rows) via one DMA each
                nc.gpsimd.dma_start(rr[D:KD, :], padq_d[:])
                nc.gpsimd.dma_start(lr[D:KD, :], padl_d[:])
                nc.gpsimd.memset(rr[0:KZ, NQ:NQP].bitcast(F32), 0.0)

                ga = ppool.tile([128, C * H], F32, tag="gplane_a", bufs=1)
                gb = ppool.tile([16, C * H], F32, tag="gplane_b", bufs=1)
                gar = ppool.tile([128, C * H], F32R, tag="gplane_ar", bufs=1)
                gbr = ppool.tile([16, C * H], F32R, tag="gplane_br", bufs=1)
                gsrc = gt_d[b].rearrange("c h w -> h c w")
                nc.sync.dma_start(ga[:], gsrc[0:128])
                nc.sync.dma_start(gb[:], gsrc[128:144])
                ceng = nc.vector if b == 0 else nc.gpsimd
                ceng.tensor_copy(gar[:], ga[:])
                ceng.tensor_copy(gbr[:], gb[:])
                psp = psA if b == 0 else psB
                downsample(b, 4, gar, gbr, d4_d[b], psp=psp,
                           pstag="psA" if b == 0 else "psB",
                           ceng=None if b == 0 else nc.vector)
                downsample(b, 2, gar, gbr, d2_d[b], psp=psp,
                           pstag="psA" if b == 0 else "psB",
                           ceng=None if b == 0 else nc.vector)
                unfold_big(gt_d[b], rr[0:D, 0:NI], eng, True, f"gt{b}")

            def prep_dsmall(b, eng):
                rr = rrs[b]
                unfold_small(d4_d[b], 4, rr[0:D, NI + 576:NQ], eng, f"d4{b}")
                unfold_small(d2_d[b], 2, rr[0:D, NI:NI + 576], eng, f"d2{b}")

            qsqs = [None] * B_LOC

            def get_qsq(b):
                if qsqs[b] is None:
                    qsqs[b] = ppool.tile([D, NQ], F32R, tag="qsq", bufs=1,
                                         name=f"qsq{b}")
                return qsqs[b]

            def qsq_sq(b, part_, use_pool):
                rr = rrs[b]
                qsq = get_qsq(b)
                lo, hi = (0, 2016) if part_ == 0 else (2016, NQ)
                if use_pool == "pool":
                    nc.gpsimd.tensor_tensor(qsq[:, lo:hi],
                                            rr[0:D, lo:hi].bitcast(F32),
                                            rr[0:D, lo:hi].bitcast(F32),
                                            op=MUL)
                elif use_pool == "dve":
                    nc.vector.tensor_tensor(qsq[:, lo:hi],
                                            rr[0:D, lo:hi].bitcast(F32),
                                            rr[0:D, lo:hi].bitcast(F32),
                                            op=MUL)
                else:
                    nc.scalar.activation(qsq[:, lo:hi],
                                         rr[0:D, lo:hi].bitcast(F32), SQ)

            def qsq_bias_row(b, part_, psp=None, pstag="psB", split=False):
                psp = psp or psB
                rr = rrs[b]
                qsq = get_qsq(b)
                lo, hi = (0, 2016) if part_ == 0 else (2016, NQ)
                for jt in range(lo // 504, hi // 504):
                    if psp is psA:
                        pt = psp.tile([128, NCH, CH], F32, tag=pstag)
                        bnp = pt[:, 0, :]
                    else:
                        bnp = psp.tile([128, CH], F32, tag=pstag)
                    nc.tensor.matmul(bnp[0:1, 0:504], neg1r[:],
                                     qsq[:, jt * 504:(jt + 1) * 504])
                    sl = rr[KZ:KZ + 1, jt * 504:(jt + 1) * 504]
                    if split and jt % 2 == 1:
                        nc.vector.tensor_copy(sl, bnp[0:1, 0:504])
                    else:
                        nc.scalar.copy(sl, bnp[0:1, 0:504])
                if debug and part_ == 1:
                    nc.sync.dma_start(dbg["rr"][b], rr[:, :].bitcast(F32))

            def qsq_bias(b, use_pool, part_, psp=None, pstag="psB"):
                qsq_sq(b, part_, use_pool)
                qsq_bias_row(b, part_, psp=psp, pstag=pstag)

            def prep_x(b, eng_shuf, eng_add):
                """p1t = unfold(x); lr rows 0:27 = p1 + p2."""
                unfold_big(x_d[b], p1ts[b][:, :], eng_shuf, False, f"x{b}")
                eng_add.tensor_tensor(lrs[b][0:D, :], p1ts[b][:, :],
                                      rrs[b][0:D, 0:NI].bitcast(F32), op=ADD)
                if debug:
                    nc.sync.dma_start(dbg["lr"][b], lrs[b][:, :].bitcast(F32))
                    nc.sync.dma_start(dbg["p1t"][b], p1ts[b][:, :])

            def main(b, hook=None):
                lr, rr = lrs[b], rrs[b]
                rv = rr[:, :].rearrange("p (k two) -> p k two", two=2)
                for t in range(NIT):
                    if hook is not None:
                        hook(t)
                    lrt = lr[:, t * IT:(t + 1) * IT]
                    scB = scpool.tile([128, HK], F32, tag="scB")
                    for c in range(NCH):
                        pb = psB.tile([128, CH], F32, tag="psB")
                        nc.tensor.matmul(pb[:, :], lrt,
                                         rv[:, CH * c:CH * (c + 1), 1])
                        nc.scalar.copy(scB[:, CH * c:CH * (c + 1)],
                                       pb[:, :])
                    pa = psA.tile([128, NCH, CH], F32, tag="psA")
                    for c in range(NCH):
                        nc.tensor.matmul(pa[:, c, :], lrt,
                                         rv[:, CH * c:CH * (c + 1), 0])
                    flatA = pa[:, :, :].rearrange("p a b -> p (a b)")
                    nc.vector._custom_dve(
                        PAIR_OP, out=junk[:], in0=flatA[:, :], in1=scB[:, :],
                        accum_out=idxf[b][:, t:t + 1], imm2=2.0,
                    )
                if debug:
                    nc.sync.dma_start(dbg["idxf"][b], idxf[b][:, :])

            HT = NIT // 2
            HNI = NI // 2

            def tail_idx(b, h):
                """PMT permute + wrapped i16 idx for tiles [h*9,(h+1)*9)."""
                wp = psB.tile([128, CH], F32, tag="psB",
                              name=f"pswp{b}_{h}")
                wp3 = wp[0:128, 0:8 * HT].rearrange("p (m t) -> p m t", m=8)
                for m in range(8):
                    nc.tensor.matmul(
                        wp3[:, m, :], pmt_t[:, m * 128:(m + 1) * 128],
                        idxf[b][:, h * HT:(h + 1) * HT],
                    )
                w3 = widxs[b][:].rearrange("p (t m) -> p t m", t=NIT)
                nc.vector.tensor_copy(
                    w3[:, h * HT:(h + 1) * HT, :],
                    wp3[:, :, :].rearrange("p m t -> p t m"))

            def tail_gather(b):
                sel = sels[b]
                nc.gpsimd.ap_gather(
                    sel[:, :].rearrange("p (n d) -> p n d", d=1),
                    rrs[b][0:32, :].bitcast(F32).rearrange(
                        "p (n d) -> p n d", d=1),
                    widxs[b][0:32, :],
                    channels=32, num_elems=NQP, d=1, num_idxs=NI,
                )
                if debug:
                    nc.sync.dma_start(dbg["sel"][b], sel[:, :])
                    nc.sync.dma_start(dbg["idxf"][b], idxf[b][:, :])

            def tail_abs(b, h):
                nc.vector._custom_dve(
                    ABS_OP, out=junk[0:D, 0:HNI],
                    in0=sels[b][0:D, h * HNI:(h + 1) * HNI],
                    in1=p1ts[b][:, h * HNI:(h + 1) * HNI],
                    accum_out=part[0:D, 2 * b + h:2 * b + h + 1],
                    imm2=0.0,
                )

            # ---------------- orchestration ----------------
            # const loads on the ACT hwdge queue so image-0's at-loads own
            # the first SP-queue HWDGE slots
            nc.scalar.dma_start(cd4a_f[:], cd4_d[0:128, :])
            nc.scalar.dma_start(cd4b_f[:], cd4_d[128:144, :])
            nc.scalar.dma_start(cd2a_f[:], cd2_d[0:128, :])
            nc.scalar.dma_start(cd2b_f[:], cd2_d[128:144, :])
            nc.scalar.dma_start(idn_t[:], idn_d[:])
            nc.scalar.dma_start(neg1_t[:], neg1_d[:])
            nc.scalar.dma_start(ones_t[:], ones_d[:])
            nc.vector.tensor_copy(cd4a[:], cd4a_f[:])
            nc.vector.tensor_copy(cd4b[:], cd4b_f[:])
            nc.vector.tensor_copy(cd2a[:], cd2a_f[:])
            nc.vector.tensor_copy(cd2b[:], cd2b_f[:])
            nc.vector.tensor_copy(neg1r[:], neg1_t[:])
            prep_head(0, nc.vector)    # image0 on DVE (head-critical)
            nc.scalar.dma_start(pmt_t[:], pmt_d[:])
            prep_x(0, nc.vector, nc.vector)
            prep_dsmall(0, nc.vector)
            qsq_bias(0, use_pool="act", part_=0, psp=psA, pstag="psA")
            qsq_bias(0, use_pool="act", part_=1, psp=psA, pstag="psA")
            prep_head(1, nc.gpsimd)    # image1 on Pool; ds copies on DVE
            prep_x(1, nc.gpsimd, nc.gpsimd)
            prep_dsmall(1, nc.gpsimd)

            main(0)
            qsq_bias(1, use_pool="dve", part_=0)
            qsq_bias(1, use_pool="dve", part_=1)
            # image-0 tail (gather overlaps main(1) on Pool)
            tail_idx(0, 0)
            tail_idx(0, 1)
            tail_gather(0)
            # image-1 idx/gather chain emitted BEFORE main(1) so the
            # scheduler prioritizes its DVE widx copies over the absdiffs
            # (they only become ready at main(1) end anyway)
            main(1)
            with tc.high_priority(offset=200):
                tail_idx(1, 0)
                tail_idx(1, 1)
            tail_gather(1)
            tail_abs(0, 0)
            tail_abs(0, 1)
            tail_abs(1, 0)
            tail_abs(1, 1)

            nc.sync.dma_start(loss_d[:], part[0:D, :])

    nc.compile()
    return nc


_NC_CACHE = None


def _get_nc():
    global _NC_CACHE
    if _NC_CACHE is None:
        _NC_CACHE = build_nc()
    return _NC_CACHE


def kernel(x: np.ndarray, gt: np.ndarray, _trace=False, _debug=False):
    x = np.ascontiguousarray(np.asarray(x, dtype=np.float32))
    gt = np.ascontiguousarray(np.asarray(gt, dtype=np.float32))
    consts = make_consts()
    nc = build_nc(debug=True) if _debug else _get_nc()
    in_maps = []
    for c in range(NCORES):
        m = {"x": x[c * B_LOC:(c + 1) * B_LOC],
             "gt": gt[c * B_LOC:(c + 1) * B_LOC]}
        m.update(consts)
        in_maps.append(m)
    res = run_bass_kernel_spmd(
        nc, in_maps, core_ids=list(range(NCORES)), trace=_trace,
        trace_cores=[0] if _trace else None,
    )
    total = sum(float(r["loss"].sum()) for r in res.results)
    out = np.asarray(np.float32(total / (B_FULL * NI * D)))
    if _trace or _debug:
        return out, res
    return out

